# revision 50
# baseline (speedup 1.0000x reference)
"""Trainium2 Bass kernel for NodeReadout: out = relu(concat([node_feature, segment_sum(edge_state, edge_dst)]) @ W + b).

Strategy (8 NeuronCores, no collectives):
  - Shard edges by DESTINATION OWNER: core c owns ~12.5k nodes (degree-
    balanced round-robin), and receives exactly the edges destined to its
    nodes. All 8 cores run one NEFF with identical shapes.
  - Host lays each core's edge features out bf16, padded-CSR, transposed:
    SBUF partitions 0:64 = features of a node's first-half edges, 64:128 =
    second-half. Within a uniform-degree sub-chunk the columns are NODE-MINOR
    ([h pair-slot blocks] x [n nodes]), so the device segment-sum is a fold
    tree of full-width packed tensor_tensor adds (bf16 -> 2x DVE mode; a
    tensor_reduce would run at 1 col/cycle with no perf mode).
  - Device: stream edge chunks (DMA alternating sync/scalar queues), fold
    each sub-chunk down to 2 blocks, final fold writes the [128, SLAB] agg
    slab; per slab a 2-matmul PSUM accumulation (W1.T@nf + [W2;W2].T@agg,
    all bf16) plus fused bias+ReLU on the scalar engine produces bf16 out.
  - node_feature is one prefetched bf16 tile; output is flushed in slab
    bands on the gpsimd queue. Per-core HBM traffic ~30MB (vs 58MB fp32).
"""

import os
import sys
import types

import numpy as np
import ml_dtypes

for _p in (
    "/root/.axon_site",
    "/root/.axon_site/_ro/trn_rl_repo",
    "/opt/trn_rl_repo",
):
    if os.path.isdir(_p) and _p not in sys.path:
        sys.path.append(_p)

N_CORES = 8
D = 64
SLAB = 512  # dense slab width (one PSUM bank of fp32)
CHUNK = int(os.environ.get("GNN_CHUNK", "8192"))  # edge-stream cols per DMA
EBUF_BUFS = int(os.environ.get("GNN_EBUFS", "6"))
OUT_BAND = int(os.environ.get("GNN_OBAND", "4"))  # slabs per output DMA
EDMA_ALT = bool(int(os.environ.get("GNN_EDMA_ALT", "1")))  # alternate edge DMA queue
PREF = int(os.environ.get("GNN_PREF", "2"))  # chunk-DMA issue lookahead
NFQ_GPSIMD = bool(int(os.environ.get("GNN_NFQ_GPSIMD", "1")))  # nf/out on SWDGE
DEG_ASC = bool(int(os.environ.get("GNN_DEG_ASC", "0")))  # ascending degree order
POOL_MIN = int(os.environ.get("GNN_POOL_MIN", str(1 << 30)))  # sub cols to offload final to Pool (disabled: slower on HW)
# Interleaved half-chunk fold chains (SUBMAX=CHUNK//2) measured SLOWER on HW:
# back-to-back DVE folds contend with DMA SBUF writes (+14% per-packet time).
SUBMAX = int(os.environ.get("GNN_SUBMAX", str(CHUNK)))  # max sub-chunk cols
TAIL_IL = bool(int(os.environ.get("GNN_TAILIL", "1")))  # interleave last-2-chunk folds
DSPLIT = bool(int(os.environ.get("GNN_DSPLIT", "0")))  # pre-fire nf matmul a chunk early
TAILQ_SC = bool(int(os.environ.get("GNN_TAILQ_SC", "0")))  # tail out-bands on scalar

BF16 = ml_dtypes.bfloat16

_last_exec_time_ns = None
_last_results = None


def _install_shims():
    """Environment fixes: antenv.axon_hooks shim (NTFF profiling), no-op
    artifact upload, and a TileContext drain patch (this container's walrus
    rejects >1 sync-wait per instruction)."""
    # -- antenv.axon_hooks shim ------------------------------------------
    try:
        import antenv.axon_hooks  # noqa: F401
    except ImportError:
        try:
            import antenv

            mod = types.ModuleType("antenv.axon_hooks")
            mod._hook = None

            def set_axon_ntff_profile_hook(h):
                mod._hook = h

            def get_axon_ntff_profile_hook():
                return mod._hook

            mod.set_axon_ntff_profile_hook = set_axon_ntff_profile_hook
            mod.get_axon_ntff_profile_hook = get_axon_ntff_profile_hook
            sys.modules["antenv.axon_hooks"] = mod
            antenv.axon_hooks = mod
            try:
                from trn_agent_boot.trn_boot import _ntff_profile_via_ctypes

                so = "/opt/axon/libaxon_pjrt.so"
                if os.path.exists(so):
                    set_axon_ntff_profile_hook(_ntff_profile_via_ctypes(so))
            except Exception:
                pass
        except Exception:
            pass
    # -- artifact upload (needs a cloud bucket; not available here) ------
    try:
        import concourse.bass_utils as bu

        bu.upload_artifacts = lambda tmpdir: "local://" + tmpdir
    except Exception:
        pass
    # -- TileContext drain: split multi-sem waits ------------------------
    import concourse.mybir as mybir
    import concourse.tile as tile_mod
    from concourse.vector_clock import ScopedClock

    if getattr(tile_mod.TileContext, "_drain_patched", False):
        return
    tile_mod.TileContext._orig_drain_and_barrier = (
        tile_mod.TileContext._drain_and_barrier
    )

    def _drain_and_barrier(self, tick_clock, wait_clock):
        nc = self.nc
        probe = nc.sync.nop(nofuse=True, hint="drain_wait_split")
        wait_clock.add_sem_waits(
            probe.ins, ScopedClock({None: tick_clock.global_clock})
        )
        waits = list(probe.ins.sync_info.on_wait)
        probe.ins.sync_info.on_wait = waits[:1]
        for w in waits[1:]:
            nop = nc.sync.nop(nofuse=True, hint="drain_wait_split")
            nop.ins.sync_info = mybir.SyncInfo(on_update=[], on_wait=[w])
        nc.sync.drain()
        nc.all_engine_barrier()
        assert self.sems is not None
        popped = nc._tile_sem_poison_stack.pop()
        assert popped is self._sem_poison
        nc.clear_and_free_semaphores(list(self.sems.allocated().values()))
        nc.all_engine_barrier()

    tile_mod.TileContext._drain_and_barrier = _drain_and_barrier
    tile_mod.TileContext._patched_drain_and_barrier = _drain_and_barrier
    tile_mod.TileContext._drain_patched = True


def _split_multiwaits(nc):
    """Walrus here allows at most ONE sync-wait per instruction: hoist extra
    waits onto preceding NoOps on the same engine."""
    import concourse.mybir as mybir

    for fn in nc.m.functions:
        for blk in fn.blocks:
            insts = blk.instructions
            new = []
            for ins in insts:
                si = getattr(ins, "sync_info", None)
                waits = list(si.on_wait) if si is not None and si.on_wait else []
                if len(waits) > 1:
                    for j, w in enumerate(waits[:-1]):
                        nop = mybir.InstNoOp(
                            name=f"{ins.name}-wsplit{j}",
                            engine=ins.engine,
                            bass_nofuse=True,
                            sync_info=mybir.SyncInfo(on_update=[], on_wait=[w]),
                        )
                        new.append(nop)
                    si.on_wait = [waits[-1]]
                new.append(ins)
            blk.instructions[:] = new


def _plan(groups):
    """Sub-chunks (uniform-degree node-minor blocks) packed into DMA chunks.
    Groups are split at node granularity so chunks fill to CHUNK cols exactly
    (a chunk only closes early when < h cols of space remain).
    sub = (col_off, h, n_nodes, slot0, gi, i0); chunk = (col_off, n_cols, [subs])."""
    chunks = []
    cur_subs, cur_off, cur_cols = [], 0, 0
    for gi, (d, n, so, co) in enumerate(groups):
        h = d // 2
        i = 0
        while i < n:
            # Cap subs at SUBMAX cols so every chunk holds >= 2 independent
            # fold chains; their emission is interleaved to hide the DVE
            # self-semaphore latency of in-place fold chains.
            take = min(n - i, (CHUNK - cur_cols) // h, max(1, SUBMAX // h))
            if take == 0:
                chunks.append((cur_off, cur_cols, cur_subs))
                cur_subs, cur_off, cur_cols = [], co + i * h, 0
                continue
            cur_subs.append((co + i * h, h, take, so + i, gi, i))
            cur_cols += take * h
            i += take
    if cur_subs:
        chunks.append((cur_off, cur_cols, cur_subs))
    return _split_last(chunks)


def _split_last(chunks, min_cols=3072):
    """Split the final chunk in two so the tail folds start mid-transfer."""
    c_off, fe, subs = chunks[-1]
    if fe <= min_cols:
        return chunks
    half = fe // 2
    a, b = [], []
    acc = 0
    for sub in subs:
        sco, h, n, s0, gi, i0 = sub
        w = h * n
        if acc >= half:
            b.append(sub)
        elif acc + w <= half:
            a.append(sub)
            acc += w
        else:
            k = (half - acc) // h
            if k < 1 or k >= n:
                a.append(sub)
                acc += w
            else:
                a.append((sco, h, k, s0, gi, i0))
                acc += k * h
                b.append((sco + k * h, h, n - k, s0 + k, gi, i0 + k))
    if not b:
        return chunks
    bfe = sum(h * n for _, h, n, _, _, _ in b)
    tail = [(c_off, acc, a), (b[0][0], bfe, b)]
    if TAIL_IL:
        # halve each tail chunk's subs so their fold chains can interleave
        tail = [(co, fe, _halve_subs(subs)) for co, fe, subs in tail]
    return chunks[:-1] + tail


def _halve_subs(subs):
    out = []
    for sco, h, n, s0, gi, i0 in subs:
        k = n // 2
        if k >= 2:
            out.append((sco, h, k, s0, gi, i0))
            out.append((sco + k * h, h, n - k, s0 + k, gi, i0 + k))
        else:
            out.append((sco, h, n, s0, gi, i0))
    return out


def _prepare(node_feature, edge_state, edge_dst, W, b):
    """Host-side shard + bf16 layout. Returns (in_maps, groups, chunks,
    NSLOT, E2, col_node, N)."""
    node_feature = np.ascontiguousarray(np.asarray(node_feature), dtype=np.float32)
    edge_state = np.ascontiguousarray(np.asarray(edge_state), dtype=np.float32)
    edge_dst = np.asarray(edge_dst).astype(np.int64)
    W16 = np.ascontiguousarray(np.asarray(W, dtype=np.float32).astype(BF16))
    b = np.asarray(b, dtype=np.float32).reshape(D, 1)

    N = node_feature.shape[0]
    # Global CSR: edges grouped by destination node.
    eid_sorted = np.argsort(edge_dst, kind="stable")
    deg = np.bincount(edge_dst, minlength=N)
    starts = np.cumsum(deg) - deg
    degp = np.maximum(2, ((deg + 1) // 2) * 2)
    # Bucket rare high degrees to multiples of 8: ~0.5% more zero-padding,
    # but far fewer tiny degree groups (fold-instruction fragments).
    degp = np.where(degp > 24, ((degp + 7) // 8) * 8, degp)

    # Degree-balanced sharding: nodes sorted by padded degree are dealt
    # round-robin to cores, so per-core degree histograms match to within 1
    # and the common group structure carries almost no cross-core padding.
    rank = np.argsort(degp, kind="stable")
    core_nodes = [rank[c::N_CORES] for c in range(N_CORES)]

    # Group order = block size ASCENDING: the fragmented tiny groups (many
    # small fold instructions) run early while the DVE has slack during the
    # stream ramp; the single biggest uniform group lands last, so the
    # post-stream tail is a handful of full-width folds + one dense band.
    all_degs = sorted(int(v) for v in np.unique(degp))
    counts = {d: int(np.count_nonzero(degp == d)) for d in all_degs}
    sized = sorted(
        all_degs,
        key=lambda d: ((counts[d] + N_CORES - 1) // N_CORES) * (d // 2),
        reverse=DEG_ASC,
    )
    groups = []  # (deg, n_nodes_per_core, slot_off, col_off)
    s_off = 0
    c_off = 0
    for d in sized:
        n = (counts[d] + N_CORES - 1) // N_CORES
        groups.append((d, n, s_off, c_off))
        s_off += n
        c_off += n * (d // 2)
    NSLOT = s_off
    E2 = c_off
    chunks = _plan(groups)
    group_subs = [[] for _ in groups]
    for _, _, csubs in chunks:
        for sub in csubs:
            group_subs[sub[4]].append(sub)

    es16 = edge_state.astype(BF16)
    nf16 = node_feature.astype(BF16)

    in_maps = []
    col_node = np.full((N_CORES, NSLOT), -1, dtype=np.int64)
    for c in range(N_CORES):
        nodes = core_nodes[c]  # global ids, ascending degp
        ndeg = degp[nodes]
        edge_tc = np.zeros((2 * D, E2), dtype=BF16)
        for gi, (d, n, so, co) in enumerate(groups):
            h = d // 2
            nodes_d = nodes[ndeg == d]
            k = len(nodes_d)
            G = np.zeros((n, 2, h, D), dtype=BF16)
            if k:
                col = starts[nodes_d][:, None] + np.arange(d)[None, :]
                valid = np.arange(d)[None, :] < deg[nodes_d][:, None]
                em = np.where(valid, eid_sorted[np.where(valid, col, 0)], -1)
                mvalid = em >= 0
                Gk = np.zeros((k, d, D), dtype=BF16)
                Gk[mvalid] = es16[em[mvalid]]
                G[:k] = Gk.reshape(k, 2, h, D)
                col_node[c, so : so + k] = nodes_d
            # node-minor blocks, one per sub-chunk of this group
            for sco, sh, take, _, _, i0 in group_subs[gi]:
                blk = G[i0 : i0 + take].transpose(1, 3, 2, 0).reshape(
                    2 * D, h * take
                )
                edge_tc[:, sco : sco + take * h] = blk
        nf_tc = np.zeros((D, NSLOT), dtype=BF16)
        vm = col_node[c] >= 0
        nf_tc[:, vm] = nf16[col_node[c][vm]].T
        in_maps.append({"edge_t": edge_tc, "nf_t": nf_tc, "W": W16, "b": b})
    return in_maps, groups, chunks, NSLOT, E2, col_node, N


def _build(groups, chunks, NSLOT, E2, for_sim=False):
    import concourse.bass as bass
    import concourse.mybir as mybir
    import concourse.tile as tile_mod
    from concourse.tile import TileContext

    if for_sim:
        # CoreSim can't digest the walrus single-wait workarounds; build
        # with the stock drain and skip the multi-wait split.
        tile_mod.TileContext._drain_and_barrier = (
            tile_mod.TileContext._orig_drain_and_barrier
        )

    f32 = mybir.dt.float32
    bf16 = mybir.dt.bfloat16
    nc = bass.Bass("TRN2", target_bir_lowering=False, debug=False)
    edge_t = nc.declare_dram_parameter("edge_t", [128, E2], bf16, isOutput=False)
    nf_t = nc.declare_dram_parameter("nf_t", [64, NSLOT], bf16, isOutput=False)
    Wp = nc.declare_dram_parameter("W", [128, D], bf16, isOutput=False)
    bp = nc.declare_dram_parameter("b", [64, 1], f32, isOutput=False)
    out_t = nc.declare_dram_parameter("out_t", [64, NSLOT], bf16, isOutput=True)

    n_slab = (NSLOT + SLAB - 1) // SLAB
    add = mybir.AluOpType.add

    with TileContext(nc) as tc:
        with (
            tc.tile_pool(name="const", bufs=1) as cpool,
            tc.tile_pool(name="big", bufs=1) as bigpool,
            tc.tile_pool(name="edges", bufs=EBUF_BUFS) as epool,
            tc.tile_pool(name="psum", bufs=4, space="PSUM") as ppool,
        ):
            # Get the edge stream moving before anything else: chunk DMAs are
            # issued PREF chunks ahead of their folds so the stream never
            # waits behind compute in a queue (head-of-line blocking).
            ebufs = {}

            def issue_chunk_dma(ci):
                c_off, fe, _ = chunks[ci]
                eb = epool.tile([128, CHUNK], bf16, tag="ebuf")
                eq = nc.scalar if (EDMA_ALT and ci % 2 == 1) else nc.sync
                eq.dma_start(out=eb[:, :fe], in_=edge_t[:, c_off : c_off + fe])
                ebufs[ci] = eb

            next_issue = 0
            while next_issue < min(PREF + 1, len(chunks)):
                issue_chunk_dma(next_issue)
                next_issue += 1

            # Matmul operands must sit at base partition 0 on this HW, so:
            # m1: lhsT=W1 [64,64], rhs=nf [64,:]; m2: lhsT=[W2;W2] [128,64],
            # rhs=agg [128,:] (sums both halves in one K=128 matmul).
            # Everything below rides the scalar HWDGE queue (the gpsimd SWDGE
            # path moves bytes at less than half the HWDGE rate).
            w1 = cpool.tile([64, D], bf16)
            nc.scalar.dma_start(out=w1[:], in_=Wp[0:64, :])
            w22 = cpool.tile([128, D], bf16)
            nc.scalar.dma_start(out=w22[0:64, :], in_=Wp[64:128, :])
            nc.scalar.dma_start(out=w22[64:128, :], in_=Wp[64:128, :])
            bt = cpool.tile([64, 1], f32)
            nc.scalar.dma_start(out=bt[:], in_=bp[:])
            nfb = bigpool.tile([64, NSLOT], bf16, name="nfb")
            nfq = nc.gpsimd if NFQ_GPSIMD else nc.scalar
            nfq.dma_start(out=nfb[:], in_=nf_t[:])
            outb = bigpool.tile([64, NSLOT], bf16, name="outb")

            aggs = [
                bigpool.tile([128, SLAB], bf16, name=f"agg{i}", tag=f"agg{i}")
                for i in range(n_slab)
            ]

            ps_tiles = {}

            def dense_start(sl):
                # nf matmul depends only on nfb, not the folds: pre-fire it
                # a chunk early so only the agg matmul + relu sit on the
                # slab-completion critical chain
                s = sl * SLAB
                n = min(SLAB, NSLOT - s)
                ps = ppool.tile(
                    [64, SLAB], f32, space="PSUM", tag="ps", name=f"ps{sl}"
                )
                nc.tensor.matmul(
                    out=ps[:, :n],
                    lhsT=w1[:],
                    rhs=nfb[:, s : s + n],
                    start=True,
                    stop=False,
                )
                ps_tiles[sl] = ps

            def dense_slab(sl):
                s = sl * SLAB
                n = min(SLAB, NSLOT - s)
                if sl not in ps_tiles:
                    dense_start(sl)
                ps = ps_tiles.pop(sl)
                nc.tensor.matmul(
                    out=ps[:, :n],
                    lhsT=w22[:],
                    rhs=aggs[sl][:, :n],
                    start=False,
                    stop=True,
                )
                nc.scalar.activation(
                    out=outb[:, s : s + n],
                    in_=ps[:, :n],
                    func=mybir.ActivationFunctionType.Relu,
                    bias=bt[:],
                )

            def emit_final(m, ebuf, lo, n, s0, big):
                # final fold (m==2) or copy (m==1, i.e. degree-2 group) into
                # the agg slab tiles, split at slab boundaries. Big subs'
                # finals go to the Pool engine: the DVE runs ~100% loaded in
                # the big-group region, while Pool has slack.
                eng = nc.gpsimd if big else nc.vector
                i = 0
                while i < n:
                    sl = (s0 + i) // SLAB
                    lc = (s0 + i) % SLAB
                    cw = min(n - i, SLAB - lc)
                    if m == 2:
                        eng.tensor_tensor(
                            out=aggs[sl][:, lc : lc + cw],
                            in0=ebuf[:, lo + i : lo + i + cw],
                            in1=ebuf[:, lo + n + i : lo + n + i + cw],
                            op=add,
                        )
                    else:
                        eng.tensor_copy(
                            out=aggs[sl][:, lc : lc + cw],
                            in_=ebuf[:, lo + i : lo + i + cw],
                        )
                    i += cw

            last_chunk_of_slab = {}
            for ci, (c_off, fe, csubs) in enumerate(chunks):
                for sco, sh, sn, ss0, _, _ in csubs:
                    for sl in range(ss0 // SLAB, (ss0 + sn - 1) // SLAB + 1):
                        last_chunk_of_slab[sl] = ci

            flushed = -1
            for ci, (c_off, fe, csubs) in enumerate(chunks):
                while next_issue < min(ci + PREF + 1, len(chunks)):
                    issue_chunk_dma(next_issue)
                    next_issue += 1
                ebuf = ebufs.pop(ci)
                # Build each sub's fold chain as thunks, then emit round-robin
                # across subs: consecutive DVE instructions come from
                # independent chains, so the self-sem wait of one chain is
                # satisfied while the other executes.
                chains = []
                for sco, sh, sn, ss0, _, _ in csubs:
                    lo = sco - c_off
                    th = []
                    m = sh
                    while m > 2:
                        k = m // 2

                        def fold(lo=lo, k=k, m=m, sn=sn, ebuf=ebuf):
                            nc.vector.tensor_tensor(
                                out=ebuf[:, lo : lo + k * sn],
                                in0=ebuf[:, lo : lo + k * sn],
                                in1=ebuf[:, lo + (m - k) * sn : lo + m * sn],
                                op=add,
                            )

                        th.append(fold)
                        m -= k
                    big = sh * sn >= POOL_MIN
                    th.append(
                        lambda m=m, ebuf=ebuf, lo=lo, sn=sn, ss0=ss0, big=big:
                        emit_final(m, ebuf, lo, sn, ss0, big)
                    )
                    chains.append(th)
                # Sequential per-sub emission (round-robin interleave across
                # chains measured slower on HW mid-stream: DVE SBUF traffic
                # contends with chunk-DMA writes). In the last two chunks the
                # stream is over, so interleaving there only hides the DVE
                # self-sem latency of the in-place chains.
                if TAIL_IL and ci >= len(chunks) - 2:
                    while chains:
                        chains = [c for c in chains if c]
                        for c in chains:
                            if c:
                                c.pop(0)()
                else:
                    for c in chains:
                        for th in c:
                            th()
                # In the last two chunks the edge stream is done: flush each
                # slab as it completes, on the now-idle sync HWDGE queue.
                at_tail = ci >= len(chunks) - 2
                band = 1 if at_tail else OUT_BAND
                # tail bands on scalar: the relu runs there, so the DMA issue
                # follows it in program order with no cross-engine sem hop
                tq = nc.scalar if TAILQ_SC else nc.sync
                outq = tq if at_tail else nfq
                for sl in sorted(
                    s for s, lc in last_chunk_of_slab.items() if lc == ci
                ):
                    dense_slab(sl)
                    if sl == n_slab - 1 or (sl - flushed) >= band:
                        a = (flushed + 1) * SLAB
                        bnd = min((sl + 1) * SLAB, NSLOT)
                        outq.dma_start(
                            out=out_t[:, a:bnd], in_=outb[:, a:bnd]
                        )
                        flushed = sl
                if DSPLIT:
                    for sl in sorted(
                        s for s, lc in last_chunk_of_slab.items() if lc == ci + 1
                    ):
                        dense_start(sl)
    if for_sim:
        # restore the patched drain for subsequent HW builds
        tile_mod.TileContext._drain_and_barrier = (
            tile_mod.TileContext._patched_drain_and_barrier
        )
    else:
        _split_multiwaits(nc)
    return nc


def kernel(node_feature, edge_state, edge_dst, W, b):
    global _last_exec_time_ns, _last_results
    _install_shims()
    from concourse.bass_utils import run_bass_kernel_spmd

    in_maps, groups, chunks, NSLOT, E2, col_node, N = _prepare(
        node_feature, edge_state, edge_dst, W, b
    )
    nc = _build(groups, chunks, NSLOT, E2)
    trace = bool(os.environ.get("GNN_TRACE"))
    res = run_bass_kernel_spmd(
        nc, in_maps, core_ids=list(range(N_CORES)), trace=trace
    )
    _last_exec_time_ns = res.exec_time_ns
    _last_results = res
    out = np.zeros((N, D), dtype=np.float32)
    for c in range(N_CORES):
        ot = np.asarray(res.results[c]["out_t"]).astype(np.float32)
        vm = col_node[c] >= 0
        out[col_node[c][vm]] = ot[:, vm].T
    return out


def last_exec_time_ns():
    return _last_exec_time_ns


def last_results():
    return _last_results
